# revision 14
# baseline (speedup 1.0000x reference)
"""TAGConv (2-layer, K=3) on 8 trn2 NeuronCores via Bass/Tile.

Design V1 (4-queue SWDGE gather + host-pregathered hop 1 + on-device M):
  - Nodes sharded: core c owns rows [c*CHUNK, (c+1)*CHUNK) of the padded node
    range NPAD = 8*CHUNK.  Edges partitioned by destination core, grouped by
    (dst-tile of 128, src-half), padded to 128-edge blocks; within each
    (tile, half) group, blocks alternate between two SWDGE queues so the
    four queue pairs (2 halves x 2 parities) generate DMA descriptors
    concurrently on the four Q7 DSP pairs.
  - Hop 1 of layer 1 gathers rows of x*dis, a pure input: the host
    pre-gathers those edge rows into a block-structured buffer streamed by
    static DMA (no descriptor generation at all for that hop).
  - Scatter M matrices built on device per block with one DVE op:
    M[e, d] = (iota[d] == p[e]) * dis[dst[e]]  (tensor_scalar, 2 scalar ops).
  - Segment sum = PE matmul B_blk^T @ M_blk accumulating in PSUM per dst
    tile (feature-major out).  h_T stashed per k; tables replicated across
    cores by AllGather of the dis-scaled node-major staging.
  - Layer output: out_T[t] = sum_k W_k^T @ h_k_T[t] in PSUM; relu/bias;
    final transpose to node-major f32.
"""

import numpy as np
import ml_dtypes
from contextlib import ExitStack

import concourse.bass as bass
import concourse.tile as tile
from concourse import bacc, mybir
from concourse.bass_utils import run_bass_kernel_spmd

F32 = mybir.dt.float32
BF16 = mybir.dt.bfloat16
I16 = mybir.dt.int16
I32 = mybir.dt.int32


class Cfg:
    def __init__(self, N, E, F=128, HID=128, C=32, K=3, NCORE=8, CHUNK=None,
                 GBLK=12, GBLK1=32, GBLKM=24):
        self.N, self.E, self.F, self.HID, self.C, self.K = N, E, F, HID, C, K
        self.NCORE = NCORE
        if CHUNK is None:
            CHUNK = ((N + NCORE - 1) // NCORE + 127) // 128 * 128
        self.CHUNK = CHUNK
        self.NPAD = NCORE * CHUNK
        self.TILES = CHUNK // 128
        self.T0 = (self.TILES + 1) // 2          # tiles in pwc-half 0
        self.T1 = self.TILES - self.T0
        self.H0 = NCORE * self.T0 * 128          # table rows in half 0
        self.H1 = NCORE * self.T1 * 128
        self.STRIPES = self.NPAD // 128
        self.GBLK = GBLK      # blocks per gather call (per queue)
        self.GBLK1 = GBLK1    # blocks per hop-1 static stream call
        self.GBLKM = GBLKM    # blocks per M stream call
        assert self.H0 <= 32768 and self.H1 <= 32768, "int16 gather index limit"


def preprocess(cfg, edge_index):
    """Host-side index reorganization.

    Edges grouped by (core, dst-tile, src-half); blocks of 128 within each
    (tile, half) group alternate between queue parities.  Block counts are
    the max over cores so the SPMD program is uniform.

    Returns per-queue idx streams (int16 wrapped), per-queue pv/disdst
    streams, hop-1 stream metadata, and the src arrays needed to build the
    hop-1 pregathered rows.
    """
    c = cfg
    src, dst = edge_index[0].astype(np.int64), edge_index[1].astype(np.int64)
    owner = dst // c.CHUNK
    t_all = (dst % c.CHUNK) >> 7
    p_all = dst & 127
    # src side: pwc (position within chunk) halves for the split AllGather
    c_s = src // c.CHUNK
    loc_s = src % c.CHUNK
    h_all = (loc_s >= c.T0 * 128).astype(np.int64)
    sidx = np.where(h_all == 0, c_s * (c.T0 * 128) + loc_s,
                    c_s * (c.T1 * 128) + loc_s - c.T0 * 128)

    deg = np.bincount(edge_index[1], minlength=c.N).astype(np.float32)
    dis = np.where(deg > 0, np.maximum(deg, 1.0) ** -0.5, 0.0).astype(np.float32)
    dis_pad = np.zeros(c.NPAD, np.float32)
    dis_pad[: c.N] = dis

    # ---- hop-1 grouping: (core, tile) only --------------------------------
    key1 = owner * c.TILES + t_all
    order1 = np.argsort(key1, kind="stable")
    cnt1 = np.bincount(key1, minlength=c.NCORE * c.TILES).reshape(c.NCORE, c.TILES)
    B1 = ((cnt1 + 127) // 128).max(axis=0)            # [TILES]
    NB1 = int(B1.sum())
    s1_off = np.concatenate([[0], np.cumsum(B1)])
    starts1 = np.zeros(c.NCORE * c.TILES + 1, np.int64)
    np.cumsum(cnt1.reshape(-1), out=starts1[1:])

    src1 = np.zeros((c.NCORE, NB1 * 128), np.int64)   # src node (global, padded range)
    pv1 = np.full((c.NCORE, NB1 * 128), 128.0, np.float32)
    dd1 = np.zeros((c.NCORE, NB1 * 128), np.float32)
    for core in range(c.NCORE):
        for t in range(c.TILES):
            g = core * c.TILES + t
            n = int(cnt1[core, t])
            a = int(starts1[g])
            sel = order1[a : a + n]
            base = int(s1_off[t]) * 128
            src1[core, base : base + n] = src[sel]
            pv1[core, base : base + n] = p_all[sel].astype(np.float32)
            dd1[core, base : base + n] = dis_pad[dst[sel]]

    # ---- hop 2+ grouping: (core, tile, half), blocks split by parity ------
    key = (owner * c.TILES + t_all) * 2 + h_all
    order = np.argsort(key, kind="stable")
    ngrp = c.NCORE * c.TILES * 2
    counts = np.bincount(key, minlength=ngrp).reshape(c.NCORE, c.TILES, 2)
    starts = np.zeros(ngrp + 1, np.int64)
    np.cumsum(counts.reshape(-1), out=starts[1:])
    Bh = ((counts + 127) // 128).max(axis=0)          # [TILES, 2]

    # per (tile, half): blocks j=0..Bh-1 -> queue 2*h + ((j + t) % 2); the
    # +t alternates which parity queue takes the odd block so loads balance
    Bq = np.zeros((c.TILES, 4), np.int64)
    for t in range(c.TILES):
        for h in range(2):
            b = int(Bh[t, h])
            Bq[t, 2 * h + (t % 2)] = (b + 1) // 2
            Bq[t, 2 * h + 1 - (t % 2)] = b // 2
    NBq = Bq.sum(axis=0)                              # [4]
    sq_off = np.zeros((c.TILES + 1, 4), np.int64)
    np.cumsum(Bq, axis=0, out=sq_off[1:])

    idxq = [np.zeros((c.NCORE, int(NBq[q]) * 128), np.int16) for q in range(4)]
    pvq = [np.full((c.NCORE, int(NBq[q]) * 128), 128.0, np.float32) for q in range(4)]
    ddq = [np.zeros((c.NCORE, int(NBq[q]) * 128), np.float32) for q in range(4)]

    for core in range(c.NCORE):
        for t in range(c.TILES):
            for h in range(2):
                g = (core * c.TILES + t) * 2 + h
                n = int(counts[core, t, h])
                a = int(starts[g])
                sel = order[a : a + n]
                lsrc = sidx[sel].astype(np.int16)
                lpv = p_all[sel].astype(np.float32)
                ldd = dis_pad[dst[sel]]
                # scatter this group's edges into parity-alternating blocks
                for j in range(int(Bh[t, h])):
                    q = 2 * h + ((j + t) % 2)
                    jq = j // 2                      # block index within queue stream
                    e0, e1 = j * 128, min((j + 1) * 128, n)
                    if e0 >= n:
                        break
                    base = (int(sq_off[t, q]) + jq) * 128
                    m = e1 - e0
                    idxq[q][core, base : base + m] = lsrc[e0:e1]
                    pvq[q][core, base : base + m] = lpv[e0:e1]
                    ddq[q][core, base : base + m] = ldd[e0:e1]

    def wrap_idx(a):
        # stream position i -> (partition i%16, col i//16), replicated x8
        m = a.reshape(a.shape[0], -1, 16)
        m = np.swapaxes(m, 1, 2)
        return np.tile(m, (1, 8, 1)).copy()

    def colmaj(a):
        # [core, NB*128] -> [core, 128, NB]: column per block, partition = pos
        return a.reshape(a.shape[0], -1, 128).transpose(0, 2, 1).copy()

    return dict(
        dis_pad=dis_pad,
        B1=B1, NB1=NB1, src1=src1, pv1=colmaj(pv1), dd1=colmaj(dd1),
        Bq=Bq, NBq=NBq,
        idxq=[wrap_idx(a) if a.shape[1] else np.zeros((c.NCORE, 128, 0), np.int16)
              for a in idxq],
        pvq=[colmaj(a) if a.shape[1] else np.zeros((c.NCORE, 128, 0), np.float32)
             for a in pvq],
        ddq=[colmaj(a) if a.shape[1] else np.zeros((c.NCORE, 128, 0), np.float32)
             for a in ddq],
    )


def _calls(total, gblk):
    out = []
    b = 0
    while b < total:
        nb = min(gblk, total - b)
        out.append((b, nb))
        b += nb
    return out


def build_nc(cfg, meta):
    c = cfg
    B1, NB1 = meta["B1"], meta["NB1"]
    Bq, NBq = meta["Bq"], [int(x) for x in meta["NBq"]]
    s1_off = np.concatenate([[0], np.cumsum(B1)])
    sq_off = np.zeros((c.TILES + 1, 4), np.int64)
    np.cumsum(Bq, axis=0, out=sq_off[1:])
    calls1 = _calls(NB1, c.GBLK1)
    callsq = [_calls(NBq[q], c.GBLK) for q in range(4)]
    NBQT_ = int(Bq.sum())
    callsM1 = _calls(NB1, c.GBLKM)
    callsM2 = _calls(NBQT_, c.GBLKM)

    nc = bacc.Bacc(None, target_bir_lowering=False, num_swdge_queues=4)

    # ---- parameters -----------------------------------------------------
    xchunk = nc.declare_dram_parameter("xchunk", [128, c.TILES, c.F], F32, isOutput=False)
    rows1_d = nc.declare_dram_parameter("rows1", [128, max(NB1, 1), c.F], BF16, isOutput=False)
    m1_d = nc.declare_dram_parameter("m1", [128, max(NB1, 1), 128], BF16, isOutput=False)
    NBQT = int(sum(NBq))
    m2_d = nc.declare_dram_parameter("m2", [128, max(NBQT, 1), 128], BF16, isOutput=False)
    idxq_d = [nc.declare_dram_parameter(f"idx{q}", [128, max(NBq[q] * 8, 1)], I16, isOutput=False)
              for q in range(4)]
    discol_d = nc.declare_dram_parameter("discol", [128, c.TILES], F32, isOutput=False)
    w1_d = nc.declare_dram_parameter("w1", [c.K + 1, c.F, c.HID], F32, isOutput=False)
    b1_d = nc.declare_dram_parameter("b1", [c.HID, 1], F32, isOutput=False)
    w2_d = nc.declare_dram_parameter("w2", [c.K + 1, c.HID, c.C], F32, isOutput=False)
    b2_d = nc.declare_dram_parameter("b2", [c.C, 1], F32, isOutput=False)
    out_d = nc.declare_dram_parameter("out", [c.CHUNK, c.C], F32, isOutput=True)

    # ---- internal DRAM --------------------------------------------------
    tabs = [nc.dram_tensor(f"table{i}", [c.NPAD, c.F], BF16, kind="Internal",
                           addr_space="Shared") for i in range(2)]
    stage_in0 = nc.dram_tensor("stage_in0", [c.T0 * 128, c.F], BF16, kind="Internal")
    stage_in1 = nc.dram_tensor("stage_in1", [c.T1 * 128, c.F], BF16, kind="Internal")

    with tile.TileContext(nc) as tc, ExitStack() as ctx:
        sp = ctx.enter_context(tc.tile_pool(name="sp", bufs=1))          # residents
        gps = [ctx.enter_context(tc.tile_pool(name=f"gp{q}", bufs=5))    # per-queue gather
               for q in range(4)]
        rp = ctx.enter_context(tc.tile_pool(name="rp", bufs=2))          # hop-1 rows stream
        mp = ctx.enter_context(tc.tile_pool(name="mp", bufs=4))          # M tiles
        wp = ctx.enter_context(tc.tile_pool(name="wp", bufs=2))          # small work tiles
        ps_seg = ctx.enter_context(tc.tile_pool(name="ps_seg", bufs=4, space="PSUM"))
        ps_tr = ctx.enter_context(tc.tile_pool(name="ps_tr", bufs=2, space="PSUM"))
        ps_w = ctx.enter_context(tc.tile_pool(name="ps_w", bufs=2, space="PSUM"))

        # ---- residents --------------------------------------------------
        idxq_t = []
        for q in range(4):
            t_ = sp.tile([128, max(NBq[q] * 8, 1)], I16, tag=f"idxq{q}", name=f"idxq{q}")
            nc.sync.dma_start(t_[:], idxq_d[q][:, :])
            idxq_t.append(t_)
        discol_t = sp.tile([128, c.TILES], F32)
        nc.sync.dma_start(discol_t[:], discol_d[:, :])
        b1_t = sp.tile([c.HID, 1], F32)
        nc.sync.dma_start(b1_t[:], b1_d[:, :])
        b2_t = sp.tile([c.C, 1], F32)
        nc.sync.dma_start(b2_t[:], b2_d[:, :])

        # weights -> bf16
        w1_t, w2_t = [], []
        for k in range(c.K + 1):
            wf = wp.tile([c.F, c.HID], F32, tag="wload", name="wload")
            nc.sync.dma_start(wf[:], w1_d[k, :, :])
            wb = sp.tile([c.F, c.HID], BF16, tag=f"w1_{k}", name=f"w1_{k}")
            nc.vector.tensor_copy(wb[:], wf[:])
            w1_t.append(wb)
        for k in range(c.K + 1):
            wf = wp.tile([c.HID, c.C], F32, tag="wload2", name="wload2")
            nc.sync.dma_start(wf[:], w2_d[k, :, :])
            wb = sp.tile([c.HID, c.C], BF16, tag=f"w2_{k}", name=f"w2_{k}")
            nc.vector.tensor_copy(wb[:], wf[:])
            w2_t.append(wb)

        # iota constants + identities
        iota_i = sp.tile([128, 128], I32)
        nc.gpsimd.iota(iota_i[:], pattern=[[1, 128]], base=0, channel_multiplier=0)
        iota_bf = sp.tile([128, 128], BF16)
        nc.vector.tensor_copy(iota_bf[:], iota_i[:])
        iota_f = sp.tile([128, 128], F32)
        nc.vector.tensor_copy(iota_f[:], iota_i[:])
        iop_i = sp.tile([128, 1], I32)
        nc.gpsimd.iota(iop_i[:], pattern=[[1, 1]], base=0, channel_multiplier=1)
        iop_f = sp.tile([128, 1], F32)
        nc.vector.tensor_copy(iop_f[:], iop_i[:])
        ident_bf = sp.tile([128, 128], BF16)
        nc.vector.tensor_scalar(ident_bf[:], iota_bf[:], iop_f[:], None,
                                mybir.AluOpType.is_equal)
        ident_f = sp.tile([128, 128], F32)
        nc.vector.tensor_scalar(ident_f[:], iota_f[:], iop_f[:], None,
                                mybir.AluOpType.is_equal)

        # stash: h_T tiles per k (feature-major bf16), stash[0] = x_T
        stash = [sp.tile([128, c.CHUNK], BF16, tag=f"stash{k}", name=f"stash{k}")
                 for k in range(c.K + 1)]
        staging = sp.tile([128, c.TILES, c.F], BF16, tag="staging", name="staging")

        # ---- x_T stash (k=0 of layer 1) ---------------------------------
        for t in range(c.TILES):
            xt = wp.tile([128, c.F], F32, tag="xT", name="xT")
            nc.sync.dma_start(xt[:], xchunk[:, t, :])
            pt = ps_tr.tile([128, 128], F32, tag="ptr", name="ptr_f")
            nc.tensor.transpose(pt[:], xt[:], ident_f[:])
            nc.vector.tensor_copy(stash[0][:, t * 128 : (t + 1) * 128], pt[:])

        def stage_half(dst_tab, half):
            if half == 0:
                sin, rows, tsl = stage_in0, slice(0, c.H0), slice(0, c.T0)
            else:
                sin, rows, tsl = stage_in1, slice(c.H0, c.NPAD), slice(c.T0, c.TILES)
            nc.sync.dma_start(
                sin[:, :].rearrange("(t p) f -> p t f", p=128),
                staging[:, tsl, :])
            nc.gpsimd.collective_compute(
                    "AllGather", mybir.AluOpType.bypass,
                    replica_groups=[list(range(c.NCORE))],
                    ins=[sin[:, :].opt()],
                    outs=[dst_tab[rows, :].opt()],
                )

        def stream_m(m_d, calls):
            out = []
            for (b, nb) in calls:
                mt = mp.tile([128, c.GBLKM, 128], BF16, tag="mstream", name="mstream")
                nc.scalar.dma_start(mt[:, 0:nb, :], m_d[:, b : b + nb, :])
                out.append((b, nb, mt))
            return out

        def stream_slot(bufs, pos):
            for (b, nb, t_) in bufs:
                if b <= pos < b + nb:
                    return t_[:, pos - b, :]
            raise AssertionError(pos)

        def finish_tile(k, t, ps, last):
            """psum [f, d] for tile t -> stash + staging."""
            tr = slice(t * 128, (t + 1) * 128)
            nc.vector.tensor_copy(stash[k][:, tr], ps[:])
            if not last:
                pt = ps_tr.tile([128, 128], BF16, tag="ptr", name="ptr_b")
                nc.tensor.transpose(pt[:], stash[k][:, tr], ident_bf[:])
                nc.vector.tensor_scalar(staging[:, t, :], pt[:],
                                        discol_t[:, t : t + 1], None,
                                        mybir.AluOpType.mult)

        # ---- hop 1: pregathered rows ------------------------------------
        def hop1(k, last, dst_tab=None):
            row_bufs = []
            for (b, nb) in calls1:
                rt = rp.tile([128, c.GBLK1, c.F], BF16, tag="r1", name="r1")
                nc.sync.dma_start(rt[:, 0:nb, :], rows1_d[:, b : b + nb, :])
                row_bufs.append((b, nb, rt))
            m_bufs = stream_m(m1_d, callsM1)
            for t in range(c.TILES):
                nb_t = int(B1[t])
                if nb_t == 0:
                    nc.vector.memset(stash[k][:, t * 128 : (t + 1) * 128], 0.0)
                    if not last:
                        nc.vector.memset(staging[:, t, :], 0.0)
                    continue
                ps = ps_seg.tile([128, 128], F32, tag="seg", name="seg")
                for j in range(nb_t):
                    pos = int(s1_off[t]) + j
                    bsl = stream_slot(row_bufs, pos)
                    msl = stream_slot(m_bufs, pos)
                    nc.tensor.matmul(ps[:], bsl, msl, start=(j == 0),
                                     stop=(j == nb_t - 1))
                finish_tile(k, t, ps, last)
                if not last and t == c.T0 - 1:
                    stage_half(dst_tab, 0)
            if not last:
                stage_half(dst_tab, 1)

        # ---- hop 2+: 4-queue gather -------------------------------------
        def hop(par, k, last, dst_tab=None):
            gat_bufs = [[] for _ in range(4)]
            ncalls = max(len(callsq[q]) for q in range(4))
            # emission order: queues 0/1 (ready after AG of table half 0) get a
            # two-call head start over queues 2/3 so a wait on the half-1 AG
            # doesn't block ready half-0 calls behind it in the engine queue
            sched = []
            for gi in range(ncalls + 4):
                for q in range(4):
                    gj = gi if q < 2 else gi - 4
                    if 0 <= gj < len(callsq[q]):
                        sched.append((q, gj))
            for q, gi in sched:
                    b, nb = callsq[q][gi]
                    h = q // 2
                    tab_rows = (tabs[par][0 : c.H0, :] if h == 0
                                else tabs[par][c.H0 : c.NPAD, :])
                    gt = gps[q].tile([128, c.GBLK, 128], BF16, tag=f"g{q}", name=f"g{q}")
                    nc.gpsimd.dma_gather(gt[:, 0:nb, :], tab_rows,
                                         idxq_t[q][:, b * 8 : (b + nb) * 8],
                                         nb * 128, nb * 128, c.F,
                                         single_packet=False, queue_num=q)
                    gat_bufs[q].append((b, nb, gt))
            m_bufs = stream_m(m2_d, callsM2)
            mpos = 0
            for t in range(c.TILES):
                nb_t = int(Bq[t].sum())
                tr = slice(t * 128, (t + 1) * 128)
                if nb_t == 0:
                    nc.vector.memset(stash[k][:, tr], 0.0)
                    if not last:
                        nc.vector.memset(staging[:, t, :], 0.0)
                    continue
                ps = ps_seg.tile([128, 128], F32, tag="seg", name="seg")
                j = 0
                for q in range(4):
                    for jq in range(int(Bq[t, q])):
                        pos = int(sq_off[t, q]) + jq
                        bsl = stream_slot(gat_bufs[q], pos)
                        msl = stream_slot(m_bufs, mpos)
                        mpos += 1
                        nc.tensor.matmul(ps[:], bsl, msl, start=(j == 0),
                                         stop=(j == nb_t - 1))
                        j += 1
                finish_tile(k, t, ps, last)
                if not last and t == c.T0 - 1:
                    stage_half(dst_tab, 0)
            if not last:
                stage_half(dst_tab, 1)

        def layer_end(layer, dst_tab=None):
            """W-matmuls from stash; layer 1 -> relu into stash[0] + staging
            + AllGather; layer 2 -> final output."""
            if layer == 1:
                fout, w_t = c.HID, w1_t
            else:
                fout, w_t = c.C, w2_t
            for t in range(c.TILES):
                tr = slice(t * 128, (t + 1) * 128)
                ps = ps_w.tile([fout, 128], F32, tag="wps", name="wps")
                for k in range(c.K + 1):
                    nc.tensor.matmul(ps[:], w_t[k][:], stash[k][:, tr],
                                     start=(k == 0), stop=(k == c.K))
                if layer == 1:
                    nc.scalar.activation(stash[0][:, tr], ps[:],
                                         mybir.ActivationFunctionType.Relu,
                                         bias=b1_t[:, 0:1])
                    pt = ps_tr.tile([128, 128], BF16, tag="ptr", name="ptr_b")
                    nc.tensor.transpose(pt[:], stash[0][:, tr], ident_bf[:])
                    nc.vector.tensor_scalar(staging[:, t, :], pt[:],
                                            discol_t[:, t : t + 1], None,
                                            mybir.AluOpType.mult)
                    if t == c.T0 - 1:
                        stage_half(dst_tab, 0)
                else:
                    o2 = wp.tile([c.C, 128], F32, tag="o2T", name="o2T")
                    nc.vector.tensor_scalar(o2[:], ps[:], b2_t[:, 0:1], None,
                                            mybir.AluOpType.add)
                    pt2 = ps_tr.tile([128, c.C], F32, tag="ptr", name="ptr_o")
                    nc.tensor.transpose(pt2[:], o2[:], ident_f[0 : c.C, 0 : c.C])
                    ot = wp.tile([128, c.C], F32, tag="ofin", name="ofin")
                    nc.vector.tensor_copy(ot[:], pt2[:])
                    nc.sync.dma_start(out_d[t * 128 : (t + 1) * 128, :], ot[:])
            if layer == 1:
                stage_half(dst_tab, 1)

        # layer 1: hop1 streams pregathered rows; AG -> tabs[0]; hop2 reads
        # tabs[0], AG -> tabs[1]; hop3 reads tabs[1] (last).
        hop1(1, False, dst_tab=tabs[0])
        hop(0, 2, False, dst_tab=tabs[1])
        hop(1, 3, True)
        layer_end(1, dst_tab=tabs[0])   # h -> stash[0]; h*dis -> tabs[0]
        # layer 2: hops read tabs 0,1,0
        hop(0, 1, False, dst_tab=tabs[1])
        hop(1, 2, False, dst_tab=tabs[0])
        hop(0, 3, True)
        layer_end(2)

    nc.finalize()
    return nc


def make_host_data(cfg, inputs):
    """Full inputs -> (meta, per-core in_maps)."""
    c = cfg
    x = np.asarray(inputs["x"], np.float32)
    ei = np.asarray(inputs["edge_index"])
    w1 = np.asarray(inputs["w1"], np.float32)
    b1 = np.asarray(inputs["b1"], np.float32)
    w2 = np.asarray(inputs["w2"], np.float32)
    b2 = np.asarray(inputs["b2"], np.float32)

    meta = preprocess(c, ei)
    dis_pad = meta["dis_pad"]

    xpad = np.zeros((c.NPAD, c.F), np.float32)
    xpad[: c.N] = x
    # [128, STRIPES, F]: xfull[p, s, f] = xpad[s*128+p, f]
    xfull = xpad.reshape(c.STRIPES, 128, c.F).transpose(1, 0, 2).copy()

    # hop-1 pregathered rows: xdis[src] in bf16, block-structured
    xdis = (xpad * dis_pad[:, None]).astype(ml_dtypes.bfloat16)
    NB1 = meta["NB1"]
    Bq = meta["Bq"]
    sq_off = np.zeros((c.TILES + 1, 4), np.int64)
    np.cumsum(Bq, axis=0, out=sq_off[1:])

    dcols = np.arange(128, dtype=np.float32)[None, None, :]

    def onehot_m(pv, dd):
        # pv, dd: [128, NB] -> M [128, NB, 128] bf16
        m = (pv[:, :, None] == dcols) * dd[:, :, None].astype(np.float32)
        return m.astype(ml_dtypes.bfloat16)

    # m2 stream order: per tile t, queues 0..3, block jq
    def build_m2(core):
        parts = []
        for t in range(c.TILES):
            for q in range(4):
                a, b = int(sq_off[t, q]), int(sq_off[t + 1, q])
                if b > a:
                    parts.append(onehot_m(meta["pvq"][q][core][:, a:b],
                                          meta["ddq"][q][core][:, a:b]))
        if not parts:
            return np.zeros((128, 1, 128), ml_dtypes.bfloat16)
        return np.concatenate(parts, axis=1)

    in_maps = []
    for core in range(c.NCORE):
        dchunk = dis_pad[core * c.CHUNK : (core + 1) * c.CHUNK]
        rows1 = xdis[meta["src1"][core]]              # [NB1*128, F] bf16
        # param layout [128, NB1, F]: rows1_p[p, b, f] = rows of edge (b*128+p)
        rows1_p = rows1.reshape(max(NB1, 1), 128, c.F).transpose(1, 0, 2).copy()
        im = dict(
            xchunk=xfull[:, core * c.TILES : (core + 1) * c.TILES, :].copy(),
            rows1=rows1_p,
            m1=onehot_m(meta["pv1"][core], meta["dd1"][core]),
            m2=build_m2(core),
            discol=dchunk.reshape(c.TILES, 128).T.copy(),
            w1=w1, b1=b1.reshape(c.HID, 1),
            w2=w2, b2=b2.reshape(c.C, 1),
        )
        for q in range(4):
            im[f"idx{q}"] = meta["idxq"][q][core]
        in_maps.append(im)
    return meta, in_maps


def run(cfg, inputs, nc=None, meta=None, in_maps=None, trace=False):
    if meta is None or in_maps is None:
        meta, in_maps = make_host_data(cfg, inputs)
    if nc is None:
        nc = build_nc(cfg, meta)
    res = run_bass_kernel_spmd(nc, in_maps, list(range(cfg.NCORE)), trace=trace)
    outs = [res.results[i]["out"] for i in range(cfg.NCORE)]
    full = np.concatenate(outs, axis=0)[: cfg.N]
    return full, res


_BUILT = {}


def kernel(x, edge_index, w1, b1, w2, b2):
    """Full-input TAGConv kernel on 8 NeuronCores; returns [50000, 32] f32."""
    inputs = dict(x=x, edge_index=edge_index, w1=w1, b1=b1, w2=w2, b2=b2)
    cfg = Cfg(N=50000, E=800000)
    meta, in_maps = make_host_data(cfg, inputs)
    key = (meta["NB1"], tuple(int(v) for v in meta["NBq"]))
    if key not in _BUILT:
        _BUILT[key] = build_nc(cfg, meta)
    out, _ = run(cfg, inputs, nc=_BUILT[key], meta=meta, in_maps=in_maps)
    return out.astype(np.float32)
